# revision 1
# baseline (speedup 1.0000x reference)
"""BitLinear (ternary-weight linear) on 8 Trainium2 NeuronCores.

Computation: out = x @ (clip(round(w/s), -1, 1) * s).T + bias, where s is
the per-output-row lower median of |w|.

Strategy
- Host side: compute the per-row scale s (exact reference semantics via
  np.partition) and the ternary weights wq in {-1, 0, 1}. The scale is
  applied in the on-device epilogue, so the matmul operand is the exact
  ternary matrix, shipped as int8 (16.8 MB instead of 67 MB fp32).
- Sharding: data-parallel over tokens. Each core owns 1024 of the 8192
  tokens; its x-shard (16.8 MB) sits resident in SBUF while the full
  ternary weight streams through once. No collectives.
- Matmul dtype: float32r (TF32-like, FP22) — full BF16-rate on the PE at
  N=512 moving dim, ~1e-4 relative error from truncating x. The ternary
  weights are exact in FP22.
- Per core: psum tiles [128 tokens x 512 features] x 8 banks accumulate
  over the 4096-deep contraction; DVE epilogue fuses scale+bias while
  copying PSUM -> SBUF.
"""

import os
import sys

import numpy as np

for _p in ("/opt/trn_rl_repo", "/opt/pypackages"):
    if os.path.isdir(_p) and _p not in sys.path:
        sys.path.append(_p)

N_CORES = 8
B, S, IN_F, OUT_F = 4, 2048, 4096, 4096
TOK = B * S                # 8192 tokens total
TPC = TOK // N_CORES       # 1024 tokens per core
KB = IN_F // 128           # 32 contraction blocks
FBW = 512                  # psum tile free width (one PSUM bank of fp32)
FB = OUT_F // FBW          # 8 feature blocks
TB = TPC // 128            # 8 token blocks per core
X_CHUNKS = 16              # DMAs used to land the resident x-shard

_CACHE = {}


def _patched_tile_context(nc):
    """TileContext subclass for this container's walrus, which rejects
    instructions carrying more than one sync-wait command. Tile's wait
    assignment (and its tail drain) can attach several; after scheduling,
    move the extras onto same-engine no-ops inserted just before the
    instruction (same program point, identical semantics)."""
    import concourse.mybir as mybir
    import concourse.tile as tile

    def _split_multi_waits(nc):
        for f in nc.m.functions:
            for blk in f.blocks:
                out = []
                changed = False
                for inst in blk.instructions:
                    si = inst.sync_info
                    waits = list(si.on_wait) if si and si.on_wait else []
                    cap = 2 if isinstance(inst, mybir.InstEventSemaphore) else 1
                    if len(waits) > cap:
                        changed = True
                        for w in waits[:-cap]:
                            nop = mybir.InstNoOp(
                                name=f"I-waitsplit-{nc.next_id()}", ins=[], outs=[]
                            )
                            nop.engine = inst.engine
                            nop.sync_info = mybir.SyncInfo(on_wait=[w], on_update=[])
                            out.append(nop)
                        inst.sync_info = mybir.SyncInfo(
                            on_wait=waits[-cap:], on_update=list(si.on_update or [])
                        )
                    out.append(inst)
                if changed:
                    blk.instructions = out

    class PatchedTileContext(tile.TileContext):
        def schedule_and_allocate(self):
            result = super().schedule_and_allocate()
            _split_multi_waits(self.nc)
            return result

    return PatchedTileContext(nc)


def _build_nc():
    import concourse.bass as bass
    import concourse.mybir as mybir

    F32 = mybir.dt.float32
    F32R = mybir.dt.float32r
    I8 = mybir.dt.int8

    nc = bass.Bass()
    xt = nc.declare_dram_parameter("xt", [128, KB * TPC], F32R, isOutput=False)
    wq8 = nc.declare_dram_parameter("wq8", [KB, 128, OUT_F], I8, isOutput=False)
    s_bc = nc.declare_dram_parameter("s_bc", [128, OUT_F], F32, isOutput=False)
    b_bc = nc.declare_dram_parameter("b_bc", [128, OUT_F], F32, isOutput=False)
    out = nc.declare_dram_parameter("out", [TPC, OUT_F], F32, isOutput=True)

    with _patched_tile_context(nc) as tc:
        with tc.tile_pool(name="xp", bufs=1) as xp, \
             tc.tile_pool(name="cp", bufs=1) as cp, \
             tc.tile_pool(name="wp", bufs=4) as wp, \
             tc.tile_pool(name="op", bufs=6) as op, \
             tc.tile_pool(name="pp", bufs=1, space="PSUM") as pp:

            xt_sb = xp.tile([128, KB * TPC], F32R)
            cw = (KB * TPC) // X_CHUNKS
            for c in range(X_CHUNKS):
                nc.sync.dma_start(
                    xt_sb[:, c * cw:(c + 1) * cw], xt[:, c * cw:(c + 1) * cw]
                )
            s_sb = cp.tile([128, OUT_F], F32, name="s_sb")
            nc.sync.dma_start(s_sb[:], s_bc[:])
            b_sb = cp.tile([128, OUT_F], F32, name="b_sb")
            nc.sync.dma_start(b_sb[:], b_bc[:])

            for fb in range(FB):
                ptiles = [
                    pp.tile([128, FBW], F32, name=f"ps{tb}", tag=f"ps{tb}")
                    for tb in range(TB)
                ]
                for kb in range(KB):
                    wt = wp.tile([128, FBW], F32R, name="wt", tag="wt")
                    nc.gpsimd.dma_start(
                        wt[:], wq8[kb, :, fb * FBW:(fb + 1) * FBW]
                    )
                    for tb in range(TB):
                        nc.tensor.matmul(
                            ptiles[tb][:],
                            lhsT=xt_sb[:, kb * TPC + tb * 128: kb * TPC + (tb + 1) * 128],
                            rhs=wt[:],
                            start=(kb == 0),
                            stop=(kb == KB - 1),
                        )
                for tb in range(TB):
                    ot = op.tile([128, FBW], F32, name="ot", tag="ot")
                    nc.vector.tensor_tensor(
                        ot[:], ptiles[tb][:],
                        s_sb[:, fb * FBW:(fb + 1) * FBW], mybir.AluOpType.mult,
                    )
                    nc.vector.tensor_tensor(
                        ot[:], ot[:],
                        b_sb[:, fb * FBW:(fb + 1) * FBW], mybir.AluOpType.add,
                    )
                    nc.sync.dma_start(
                        out[tb * 128:(tb + 1) * 128, fb * FBW:(fb + 1) * FBW],
                        ot[:],
                    )
    return nc


def _get_nc():
    if "nc" not in _CACHE:
        _CACHE["nc"] = _build_nc()
    return _CACHE["nc"]


def kernel(x, weight, bias):
    from concourse.bass_utils import run_bass_kernel_spmd

    x = np.asarray(x, dtype=np.float32)
    weight = np.asarray(weight, dtype=np.float32)
    bias = np.asarray(bias, dtype=np.float32)

    # Ternary quantization (matches the reference bit-for-bit): per-row
    # lower median of |w|, floored at 1e-12; wq = clip(round(w/s), -1, 1).
    mid = (IN_F - 1) // 2
    s = np.partition(np.abs(weight), mid, axis=1)[:, mid]
    s = np.maximum(s, np.float32(1e-12)).astype(np.float32)
    wq = np.clip(np.round(weight / s[:, None]), -1.0, 1.0).astype(np.int8)

    wq_h = np.ascontiguousarray(wq.T).reshape(KB, 128, OUT_F)
    s_h = np.ascontiguousarray(np.broadcast_to(s, (128, OUT_F)))
    b_h = np.ascontiguousarray(np.broadcast_to(bias, (128, OUT_F)))

    x2 = x.reshape(TOK, IN_F)
    in_maps = []
    for c in range(N_CORES):
        xs = x2[c * TPC:(c + 1) * TPC]
        xt_h = np.ascontiguousarray(
            xs.reshape(TPC, KB, 128).transpose(2, 1, 0)
        ).reshape(128, KB * TPC)
        in_maps.append({"xt": xt_h, "wq8": wq_h, "s_bc": s_h, "b_bc": b_h})

    res = run_bass_kernel_spmd(_get_nc(), in_maps, core_ids=list(range(N_CORES)))
    _CACHE["last_results"] = res
    out = np.concatenate([res.results[c]["out"] for c in range(N_CORES)], axis=0)
    return out.reshape(B, S, OUT_F)


# revision 5
# speedup vs baseline: 24206.9726x; 24206.9726x over previous
"""BitLinear (ternary-weight linear) on 8 Trainium2 NeuronCores.

Computation: out = x @ (clip(round(w/s), -1, 1) * s).T + bias, where s is
the per-output-row lower median of |w|.

Strategy
- Host side: compute the per-row scale s (exact reference semantics via
  np.partition) and the ternary weights wq in {-1, 0, 1}. The scale is
  applied in the on-device epilogue, so the matmul operand is the exact
  ternary matrix, shipped as int8 (16.8 MB instead of 67 MB fp32).
- Sharding: data-parallel over tokens. Each core owns 1024 of the 8192
  tokens; its x-shard (16.8 MB) sits resident in SBUF while the full
  ternary weight streams through once. No collectives.
- Matmul dtype: float32r (TF32-like, FP22) — full BF16-rate on the PE at
  N=512 moving dim, ~1e-4 relative error from truncating x. The ternary
  weights are exact in FP22.
- Per core: psum tiles [128 tokens x 512 features] x 8 banks accumulate
  over the 4096-deep contraction; DVE epilogue fuses scale+bias while
  copying PSUM -> SBUF.
"""

import os
import sys

import numpy as np

for _p in ("/opt/trn_rl_repo", "/opt/pypackages"):
    if os.path.isdir(_p) and _p not in sys.path:
        sys.path.append(_p)

N_CORES = 8
B, S, IN_F, OUT_F = 4, 2048, 4096, 4096
TOK = B * S                # 8192 tokens total
TPC = TOK // N_CORES       # 1024 tokens per core
KB = IN_F // 128           # 32 contraction blocks
FBW = 512                  # psum tile free width (one PSUM bank of fp32)
FB = OUT_F // FBW          # 8 feature blocks
TB = TPC // 128            # 8 token blocks per core
X_CHUNKS = 32              # DMAs used to land the resident x-shard

_CACHE = {}


def _patched_tile_context(nc):
    """TileContext subclass for this container's walrus, which rejects
    instructions carrying more than one sync-wait command. Tile's wait
    assignment (and its tail drain) can attach several; after scheduling,
    move the extras onto same-engine no-ops inserted just before the
    instruction (same program point, identical semantics)."""
    import concourse.mybir as mybir
    import concourse.tile as tile

    def _split_multi_waits(nc):
        for f in nc.m.functions:
            for blk in f.blocks:
                out = []
                changed = False
                for inst in blk.instructions:
                    si = inst.sync_info
                    waits = list(si.on_wait) if si and si.on_wait else []
                    cap = 2 if isinstance(inst, mybir.InstEventSemaphore) else 1
                    if len(waits) > cap:
                        changed = True
                        for w in waits[:-cap]:
                            nop = mybir.InstNoOp(
                                name=f"I-waitsplit-{nc.next_id()}", ins=[], outs=[]
                            )
                            nop.engine = inst.engine
                            nop.sync_info = mybir.SyncInfo(on_wait=[w], on_update=[])
                            out.append(nop)
                        inst.sync_info = mybir.SyncInfo(
                            on_wait=waits[-cap:], on_update=list(si.on_update or [])
                        )
                    out.append(inst)
                if changed:
                    blk.instructions = out

    class PatchedTileContext(tile.TileContext):
        def schedule_and_allocate(self):
            result = super().schedule_and_allocate()
            _split_multi_waits(self.nc)
            return result

    return PatchedTileContext(nc)


def _build_nc():
    import concourse.bass as bass
    import concourse.mybir as mybir

    F32 = mybir.dt.float32
    F32R = mybir.dt.float32r
    I8 = mybir.dt.int8

    nc = bass.Bass()
    xt = nc.declare_dram_parameter("xt", [128, KB * TPC], F32R, isOutput=False)
    wq8 = nc.declare_dram_parameter("wq8", [KB, 128, OUT_F], I8, isOutput=False)
    s_bc = nc.declare_dram_parameter("s_bc", [128, OUT_F], F32, isOutput=False)
    b_bc = nc.declare_dram_parameter("b_bc", [128, OUT_F], F32, isOutput=False)
    out = nc.declare_dram_parameter("out", [TPC, OUT_F], F32, isOutput=True)

    with _patched_tile_context(nc) as tc:
        with tc.tile_pool(name="xp", bufs=1) as xp, \
             tc.tile_pool(name="cp", bufs=1) as cp, \
             tc.tile_pool(name="wp", bufs=12) as wp, \
             tc.tile_pool(name="op", bufs=6) as op, \
             tc.tile_pool(name="pp", bufs=1, space="PSUM") as pp:

            xt_sb = xp.tile([128, KB * TPC], F32R)
            cw = (KB * TPC) // X_CHUNKS
            for c in range(X_CHUNKS):
                nc.sync.dma_start(
                    xt_sb[:, c * cw:(c + 1) * cw], xt[:, c * cw:(c + 1) * cw]
                )
            s_sb = cp.tile([128, OUT_F], F32, name="s_sb")
            nc.sync.dma_start(s_sb[:], s_bc[:])
            b_sb = cp.tile([128, OUT_F], F32, name="b_sb")
            nc.sync.dma_start(b_sb[:], b_bc[:])

            for fb in range(FB):
                ptiles = [
                    pp.tile([128, FBW], F32, name=f"ps{tb}", tag=f"ps{tb}")
                    for tb in range(TB)
                ]
                for kb in range(KB):
                    wt = wp.tile([128, FBW], F32R, name="wt", tag="wt")
                    nc.gpsimd.dma_start(
                        wt[:], wq8[kb, :, fb * FBW:(fb + 1) * FBW]
                    )
                    for tb in range(TB):
                        nc.tensor.matmul(
                            ptiles[tb][:],
                            lhsT=xt_sb[:, kb * TPC + tb * 128: kb * TPC + (tb + 1) * 128],
                            rhs=wt[:],
                            start=(kb == 0),
                            stop=(kb == KB - 1),
                        )
                for tb in range(TB):
                    ot = op.tile([128, FBW], F32, name="ot", tag="ot")
                    nc.vector.tensor_tensor(
                        ot[:], ptiles[tb][:],
                        s_sb[:, fb * FBW:(fb + 1) * FBW], mybir.AluOpType.mult,
                    )
                    nc.vector.tensor_tensor(
                        ot[:], ot[:],
                        b_sb[:, fb * FBW:(fb + 1) * FBW], mybir.AluOpType.add,
                    )
                    nc.sync.dma_start(
                        out[tb * 128:(tb + 1) * 128, fb * FBW:(fb + 1) * FBW],
                        ot[:],
                    )
    return nc


def _get_nc():
    if "nc" not in _CACHE:
        _CACHE["nc"] = _build_nc()
    return _CACHE["nc"]


def _get_runner():
    """Jitted SPMD executor for the prebuilt Bass module, traced once and
    cached. Mirrors concourse.bass2jax.run_bass_via_pjrt's multi-core
    path, but reusable across calls: inputs are global arrays sharded on
    axis 0 over the 8 cores; output zero-buffers are generated on-device
    and donated."""
    if "runner" in _CACHE:
        return _CACHE["runner"]
    import jax
    import jax.numpy as jnp
    from jax.experimental.shard_map import shard_map
    from jax.sharding import Mesh, NamedSharding, PartitionSpec

    import concourse.mybir as mybir
    from concourse import bass2jax

    nc = _get_nc()
    assert nc.dbg_addr is None
    bass2jax.install_neuronx_cc_hook()

    partition_name = (
        nc.partition_id_tensor.name if nc.partition_id_tensor else None
    )
    in_names, out_names, out_avals = [], [], []
    for alloc in nc.m.functions[0].allocations:
        if not isinstance(alloc, mybir.MemoryLocationSet):
            continue
        name = alloc.memorylocations[0].name
        if alloc.kind == "ExternalInput":
            if name != partition_name:
                in_names.append(name)
        elif alloc.kind == "ExternalOutput":
            out_names.append(name)
            out_avals.append(
                jax.core.ShapedArray(
                    tuple(alloc.tensor_shape), mybir.dt.np(alloc.dtype)
                )
            )
    n_params, n_outs = len(in_names), len(out_names)
    all_in_names = tuple(
        in_names + out_names + ([partition_name] if partition_name else [])
    )

    def _body(*args):
        operands = list(args)
        if partition_name is not None:
            operands.append(bass2jax.partition_id_tensor())
        outs = bass2jax._bass_exec_p.bind(
            *operands,
            out_avals=tuple(out_avals),
            in_names=all_in_names,
            out_names=tuple(out_names),
            lowering_input_output_aliases=(),
            sim_require_finite=True,
            sim_require_nnan=True,
            nc=nc,
        )
        return tuple(outs)

    devices = jax.devices()[:N_CORES]
    mesh = Mesh(np.asarray(devices), ("core",))
    sharding = NamedSharding(mesh, PartitionSpec("core"))
    in_specs = (PartitionSpec("core"),) * (n_params + n_outs)
    out_specs = (PartitionSpec("core"),) * n_outs
    donate = tuple(range(n_params, n_params + n_outs))
    sharded = jax.jit(
        shard_map(
            _body, mesh=mesh, in_specs=in_specs, out_specs=out_specs,
            check_rep=False,
        ),
        donate_argnums=donate,
        keep_unused=True,
    )
    zeros_fn = jax.jit(
        lambda: tuple(
            jnp.zeros((N_CORES * a.shape[0], *a.shape[1:]), a.dtype)
            for a in out_avals
        ),
        out_shardings=(sharding,) * n_outs,
    )
    runner = dict(
        in_names=in_names, out_names=out_names, sharded=sharded,
        zeros_fn=zeros_fn, sharding=sharding,
    )
    _CACHE["runner"] = runner
    return runner


def _prep_inputs(x, weight, bias):
    """Host-side quantization, layout, and per-core sharding. Returns the
    global (axis-0 core-sharded) input arrays in runner order."""
    x = np.asarray(x, dtype=np.float32)
    weight = np.asarray(weight, dtype=np.float32)
    bias = np.asarray(bias, dtype=np.float32)

    # Ternary quantization (matches the reference bit-for-bit): per-row
    # lower median of |w|, floored at 1e-12; wq = clip(round(w/s), -1, 1).
    mid = (IN_F - 1) // 2
    s = np.partition(np.abs(weight), mid, axis=1)[:, mid]
    s = np.maximum(s, np.float32(1e-12)).astype(np.float32)
    wq = np.clip(np.round(weight / s[:, None]), -1.0, 1.0).astype(np.int8)

    wq_h = np.ascontiguousarray(wq.T).reshape(KB, 128, OUT_F)
    s_h = np.ascontiguousarray(np.broadcast_to(s, (128, OUT_F)))
    b_h = np.ascontiguousarray(np.broadcast_to(bias, (128, OUT_F)))

    # x-shard per core, laid out [partition=i%128, kb*TPC + t]
    x4 = np.asarray(x).reshape(N_CORES, TPC, KB, 128)
    xt_all = np.ascontiguousarray(x4.transpose(0, 3, 2, 1)).reshape(
        N_CORES * 128, KB * TPC
    )
    per_name = {
        "xt": xt_all,
        "wq8": np.broadcast_to(wq_h, (N_CORES, KB, 128, OUT_F)).reshape(
            N_CORES * KB, 128, OUT_F
        ),
        "s_bc": np.broadcast_to(s_h, (N_CORES, 128, OUT_F)).reshape(
            N_CORES * 128, OUT_F
        ),
        "b_bc": np.broadcast_to(b_h, (N_CORES, 128, OUT_F)).reshape(
            N_CORES * 128, OUT_F
        ),
    }
    runner = _get_runner()
    return [np.ascontiguousarray(per_name[n]) for n in runner["in_names"]]


def _execute(dev_or_np_inputs):
    runner = _get_runner()
    zeros = runner["zeros_fn"]()
    outs = runner["sharded"](*dev_or_np_inputs, *zeros)
    return outs


def kernel(x, weight, bias):
    global_inputs = _prep_inputs(x, weight, bias)
    outs = _execute(global_inputs)
    out_name_idx = _get_runner()["out_names"].index("out")
    out = np.asarray(outs[out_name_idx])  # [TOK, OUT_F], token-sharded
    return out.reshape(B, S, OUT_F)
